# revision 1
# baseline (speedup 1.0000x reference)
"""Trainium2 Bass kernel for the AMTCL loss (nn_AMTCL_66520453480770).

Math: the reference builds a [B,B] pairwise distance matrix dist[i,j] between
inputs[i] and centers[targets[j]] (weights 2**centers_weights[targets[j]]).
Since dist[i,j] depends on j only through c = targets[j], the whole problem
collapses to the [B,C] matrix
    dc2[i,c] = sum_d w2[c,d] * (centers[c,d] - inputs[i,d])**2
with
    dist_ap[i] = sqrt(dc2[i, t_i])            (all same-class j are equal)
    dist_an[i] = sqrt(min_{c present, c != t_i} dc2[i,c])   (sqrt commutes
                 with min, so mining happens on squared distances)
    cc[i]      = centers_dist[t_i],  centers_dist[c] = sqrt(min_{j!=c} cd2[c,j])
    loss       = mean(dist_ap + relu(cc - dist_an))
This is exactly equal to the reference (40x less compute than the B^2 form);
GEMMs run in bf16 with fp32 PSUM accumulation (loss rel err ~1e-4).

dc2 is one GEMM with contraction K = 2D+1:
    dc2[i,c] = sum_d xsq[i,d]*w2[c,d] + sum_d x[i,d]*(-2*w2[c,d]*c[c,d]) + a[c]
The a[c] row rides in as a K=1 term; the cd2 GEMM shares the same center-side
operands and adds 2^40*I via an identity matmul to mask the diagonal.
Mining reads dc2 straight out of PSUM per 128-anchor chunk, overlapped with
the next chunk's matmuls.

Sharding: data-parallel over the 4096 anchor rows -> 8 cores x 512 rows.
centers/centers_weights replicated. Each core emits a partial loss sum [1,1];
the host sums the 8 scalars and divides by B.
"""

import ml_dtypes
import numpy as np

import concourse.bass as bass
import concourse.bacc as bacc
import concourse.mybir as mybir
import concourse.tile as tile
from concourse.bass_utils import run_bass_kernel_spmd

B, C, D = 4096, 100, 384
NCORES = 8
ROWS = B // NCORES          # 512 anchor rows per core
MCH = ROWS // 128           # 4 partition chunks of anchor rows
KD = D // 128               # 3 partition chunks of the feature dim
PEN = float(2 ** 40)        # self-class / absent-class / diagonal penalty
LN2 = float(np.log(2.0))
F32 = mybir.dt.float32
BF16 = mybir.dt.bfloat16
AF = mybir.ActivationFunctionType
ALU = mybir.AluOpType

# bf16 input block layout (columns); iota/eye/ones built on device.
# cwt first: it alone gates the Exp -> smaller first DMA lands sooner.
CWT_O = 0                    # centers_weights.T chunks (gates the Exp)
CT_O = CWT_O + KD * C        # centers.T chunks [128, 3*100]
T_O = CT_O + KD * C          # targets column-chunks [128, MCH]
XT_O = T_O + MCH             # x.T shard, anchor-chunk-major
BW = XT_O + KD * ROWS        # 2140

# f32 input row: absent-class penalty row (normally all zeros)
FW = C


def build_nc() -> bass.Bass:
    nc = bacc.Bacc(
        "TRN2", target_bir_lowering=False, debug=False, num_devices=NCORES
    )

    bin_ = nc.declare_dram_parameter("bin", [128, BW], BF16, isOutput=False)
    fin = nc.declare_dram_parameter("fin", [1, FW], F32, isOutput=False)
    out = nc.declare_dram_parameter("out", [1, 1], F32, isOutput=True)

    with tile.TileContext(nc) as tc:
        with (
            tc.tile_pool(name="wts", bufs=1) as wp,
            tc.tile_pool(name="work", bufs=2) as kp,
            tc.tile_pool(name="ps1", bufs=1, space="PSUM") as pp1,
            tc.tile_pool(name="ps2", bufs=3, space="PSUM") as pp2,
        ):
            # consts+centers land first (unblock prologue); x in 3 chunks.
            # Each dma_start's descriptor generation serializes on its
            # issuing sequencer (~2-3us for a [128,*] transfer), so spread
            # the loads across idle engines' DGE paths to issue in parallel.
            bsb = wp.tile([128, XT_O], BF16, tag="bsb")
            nc.sync.dma_start(bsb[:], bin_[:, 0:XT_O])
            fsb = wp.tile([1, FW], F32, tag="fsb")
            nc.sync.dma_start(fsb[:], fin[:])
            # x arrives per ANCHOR chunk (m-major): chunk m's GEMM only
            # waits for its own 98KB, not the whole shard
            xtiles = []
            for m in range(MCH):
                xm = wp.tile([128, KD * 128], BF16, tag=f"xm{m}")
                nc.sync.dma_start(
                    xm[:], bin_[:, XT_O + m * KD * 128 :
                                 XT_O + (m + 1) * KD * 128]
                )
                xtiles.append(xm)

            cwt_b = bsb[:, CWT_O : CWT_O + KD * C]
            ct_b = bsb[:, CT_O : CT_O + KD * C]
            t_b = bsb[:, T_O : T_O + MCH]
            penrow_f = fsb[0:1, 0:C]

            # ---- constants built on device (gpsimd is idle early) ----
            ones_b = wp.tile([128, 128], BF16, tag="ones_b")
            nc.gpsimd.memset(ones_b[:], 1.0)
            ones_f = wp.tile([128, 1], F32, tag="ones_f")
            nc.gpsimd.memset(ones_f[:], 1.0)
            iota_i = wp.tile([128, C], mybir.dt.int32, tag="iota_i")
            nc.gpsimd.iota(iota_i[:], pattern=[[1, C]], base=0,
                           channel_multiplier=0)
            iota_b = wp.tile([128, C], BF16, tag="iota_b")
            nc.gpsimd.tensor_copy(iota_b[:], iota_i[:])
            eye_b = wp.tile([C, C], BF16, tag="eye_b")
            nc.gpsimd.affine_select(
                eye_b[:], ones_b[0:C, 0:C], pattern=[[1, C]],
                compare_op=ALU.is_equal, fill=0.0, base=0,
                channel_multiplier=-1,
            )
            penfull = wp.tile([C, C], BF16, tag="penfull")
            nc.gpsimd.memset(penfull[:], PEN)
            eyepen_b = wp.tile([C, C], BF16, tag="eyepen_b")
            nc.gpsimd.affine_select(
                eyepen_b[:], penfull[:], pattern=[[1, C]],
                compare_op=ALU.is_equal, fill=0.0, base=0,
                channel_multiplier=-1,
            )

            # PE warm-up: HAM clock-gates a cold PE to 1.2GHz and needs
            # ~3.4us of sustained activity to ungate. Burn dummy matmuls on
            # the all-ones tile during the otherwise-dead DMA wait so the
            # real GEMMs run at 2.4GHz.
            warm_ps = pp1.tile([128, C], F32, tag="loss")
            for _ in range(24):
                nc.tensor.matmul(warm_ps[:], ones_b[:], ones_b[:, 0:C])

            # ---- center-side prep (bf16) ----
            w2b = wp.tile([128, KD * C], BF16, tag="w2b")
            nc.scalar.activation(w2b[:], cwt_b, AF.Exp, scale=LN2)
            csqb = wp.tile([128, KD * C], BF16, tag="csqb")
            nc.scalar.square(csqb[:], ct_b)
            cm2b = wp.tile([128, KD * C], BF16, tag="cm2b")
            nc.vector.tensor_scalar(cm2b[:], ct_b, -2.0, None, op0=ALU.mult)
            m2b = wp.tile([128, KD * C], BF16, tag="m2b")
            nc.vector.tensor_tensor(m2b[:], w2b[:], cm2b[:], op=ALU.mult)
            wsqb = wp.tile([128, KD * C], BF16, tag="wsqb")
            nc.vector.tensor_tensor(wsqb[:], w2b[:], csqb[:], op=ALU.mult)

            # one-hot masks (only need iota/t -> very early)
            ohw = wp.tile([128, MCH * C], F32, tag="ohw")
            oh3 = ohw[:].rearrange("p (m c) -> p m c", c=C)
            nc.vector.tensor_tensor(
                oh3, iota_b[:, None, :].broadcast_to([128, MCH, C]),
                t_b[:, :, None].broadcast_to([128, MCH, C]), op=ALU.is_equal
            )
            ohpw = wp.tile([128, MCH * C], F32, tag="ohpw")
            nc.vector.tensor_scalar(ohpw[:], ohw[:], PEN, None, op0=ALU.mult)

            # x^2 per anchor chunk (pipelines with the x DMA chunks).
            # Emitted early so the ACT engine runs these right after the
            # Exp, before the sqrt-table load. The last chunk's square runs
            # on DVE (slower per-op due to the same-tensor read-port
            # conflict, but it starts in DVE's idle window instead of
            # queueing fourth on ACT).
            xsqtiles = []
            for m in range(MCH):
                xsq = wp.tile([128, KD * 128], BF16, tag=f"xsq{m}")
                if m == MCH - 1:
                    nc.vector.tensor_tensor(
                        xsq[:], xtiles[m][:], xtiles[m][:], op=ALU.mult
                    )
                else:
                    nc.scalar.square(xsq[:], xtiles[m][:])
                xsqtiles.append(xsq)
            # dummy sqrt: triggers the sqrt ACT-table load right after the
            # last ACT square, off the cd-chain critical path
            sqdummy = wp.tile([1, 1], F32, tag="sqdummy")
            nc.scalar.sqrt(sqdummy[:], xsqtiles[MCH - 2][0:1, 0:1])

            # a[c] = sum_d w2*c^2, + absent-class penalty row -> bf16
            psum_arow = pp1.tile([1, C], F32, tag="arow")
            for k in range(KD):
                nc.tensor.matmul(
                    psum_arow[:], ones_b[:, 0:1],
                    wsqb[:, k * C : (k + 1) * C],
                    start=(k == 0), stop=(k == KD - 1),
                )
            arowb = wp.tile([1, C], BF16, tag="arowb")
            nc.vector.tensor_tensor(
                arowb[:], psum_arow[:], penrow_f, op=ALU.add
            )

            # ---- cd2 GEMM [100,100]: shared center operands + PEN*I ----
            psum_cd2 = pp1.tile([C, C], F32, tag="cd2")
            for k in range(KD):
                nc.tensor.matmul(
                    psum_cd2[:], m2b[:, k * C : (k + 1) * C],
                    ct_b[:, k * C : (k + 1) * C],
                    start=(k == 0), stop=False,
                )
                nc.tensor.matmul(
                    psum_cd2[:], w2b[:, k * C : (k + 1) * C],
                    csqb[:, k * C : (k + 1) * C],
                    start=False, stop=False,
                )
            nc.tensor.matmul(
                psum_cd2[:], eyepen_b[:], eye_b[:],
                start=False, stop=False,
            )
            nc.tensor.matmul(
                psum_cd2[:], arowb[:], ones_b[0:1, 0:C],
                start=False, stop=True,
            )
            # min over j, then clip at 0 (max is monotone, so this equals
            # the reference's clip-then-min) -- tiny [C,1] clip
            cdmin2 = wp.tile([C, 1], F32, tag="cdmin2")
            nc.vector.tensor_reduce(
                cdmin2[:], psum_cd2[:], axis=mybir.AxisListType.X, op=ALU.min
            )
            cdmin2c = wp.tile([C, 1], F32, tag="cdmin2c")
            nc.vector.tensor_scalar(cdmin2c[:], cdmin2[:], 0.0, None,
                                    op0=ALU.max)
            cdminb = wp.tile([C, 1], BF16, tag="cdminb")
            nc.scalar.sqrt(cdminb[:], cdmin2c[:])
            # transpose to a row, broadcast down 128 partitions
            psum_cdrow = pp1.tile([1, C], F32, tag="cdrow")
            nc.tensor.matmul(psum_cdrow[:], cdminb[:], eye_b[:])
            cdrowb = wp.tile([1, C], BF16, tag="cdrowb")
            nc.vector.tensor_copy(cdrowb[:], psum_cdrow[:])
            psum_bc = pp1.tile([128, C], F32, tag="bcast")
            nc.tensor.matmul(psum_bc[:], ones_b[0:1, :], cdrowb[:])
            cdbf = wp.tile([128, C], F32, tag="cdbf")
            nc.vector.tensor_copy(cdbf[:], psum_bc[:])

            # cc[i] = centers_dist[t_i] (dc2-independent -> overlapped)
            cctw = kp.tile([128, MCH * C], F32, tag="cctw")
            nc.vector.tensor_tensor(
                cctw[:].rearrange("p (m c) -> p m c", c=C),
                cdbf[:, None, :].broadcast_to([128, MCH, C]), oh3,
                op=ALU.mult,
            )
            cc4 = kp.tile([128, MCH], F32, tag="cc4")
            nc.vector.tensor_reduce(
                cc4[:], cctw[:].rearrange("p (m c) -> p m c", c=C),
                axis=mybir.AxisListType.X, op=ALU.add,
            )

            # ---- big GEMM + per-chunk mining straight out of PSUM ----
            an2 = kp.tile([128, MCH], F32, tag="an2")
            ap2 = kp.tile([128, MCH], F32, tag="ap2")
            for m in range(MCH):
                psum_dc2 = pp2.tile([128, C], F32, tag="dc2")
                for k in range(KD):
                    nc.tensor.matmul(
                        psum_dc2[:],
                        xtiles[m][:, k * 128 : (k + 1) * 128],
                        m2b[:, k * C : (k + 1) * C],
                        start=(k == 0), stop=False,
                    )
                for k in range(KD):
                    nc.tensor.matmul(
                        psum_dc2[:],
                        xsqtiles[m][:, k * 128 : (k + 1) * 128],
                        w2b[:, k * C : (k + 1) * C],
                        start=False, stop=False,
                    )
                nc.tensor.matmul(
                    psum_dc2[:], ones_b[0:1, :], arowb[:],
                    start=False, stop=True,
                )
                antm = kp.tile([128, C], F32, tag="antm")
                nc.vector.tensor_tensor(
                    antm[:], psum_dc2[:], ohpw[:, m * C : (m + 1) * C],
                    op=ALU.add,
                )
                nc.vector.tensor_reduce(
                    an2[:, m : m + 1], antm[:],
                    axis=mybir.AxisListType.X, op=ALU.min,
                )
                aptm = kp.tile([128, C], F32, tag="aptm")
                nc.vector.tensor_tensor(
                    aptm[:], psum_dc2[:], ohw[:, m * C : (m + 1) * C],
                    op=ALU.mult,
                )
                nc.vector.tensor_reduce(
                    ap2[:, m : m + 1], aptm[:],
                    axis=mybir.AxisListType.X, op=ALU.add,
                )

            # ---- loss_i = sqrt(ap2) + relu(cc - sqrt(an2)) ----
            an = kp.tile([128, MCH], F32, tag="an")
            nc.scalar.sqrt(an[:], an2[:])
            mrgin = kp.tile([128, MCH], F32, tag="mrgin")
            nc.vector.tensor_tensor(mrgin[:], cc4[:], an[:], op=ALU.subtract)
            mrg = kp.tile([128, MCH], F32, tag="mrg")
            relusum = kp.tile([128, 1], F32, tag="relusum")
            nc.scalar.activation(mrg[:], mrgin[:], AF.Relu,
                                 accum_out=relusum[:])
            ap = kp.tile([128, MCH], F32, tag="ap")
            apsum = kp.tile([128, 1], F32, tag="apsum")
            nc.scalar.activation(ap[:], ap2[:], AF.Sqrt, accum_out=apsum[:])
            losscol = kp.tile([128, 1], F32, tag="losscol")
            nc.vector.tensor_tensor(
                losscol[:], relusum[:], apsum[:], op=ALU.add
            )

            psum_loss = pp1.tile([1, 1], F32, tag="loss")
            nc.tensor.matmul(psum_loss[:], ones_f[:], losscol[:])
            res_sb = wp.tile([1, 1], F32, tag="res")
            nc.vector.tensor_copy(res_sb[:], psum_loss[:])
            nc.sync.dma_start(out[:], res_sb[:])

    nc.compile()
    return nc


_NC_CACHE: list = []


def _get_nc() -> bass.Bass:
    if not _NC_CACHE:
        _NC_CACHE.append(build_nc())
    return _NC_CACHE[0]


def make_in_maps(inputs, centers, centers_weights, targets):
    x = np.asarray(inputs, dtype=np.float32)
    c = np.asarray(centers, dtype=np.float32)
    cw = np.asarray(centers_weights, dtype=np.float32)
    t = np.asarray(targets).astype(np.int64)

    bconst = np.zeros((128, BW), dtype=np.float32)
    cT = c.T.reshape(KD, 128, C)
    cwT = cw.T.reshape(KD, 128, C)
    for k in range(KD):
        bconst[:, CT_O + k * C : CT_O + (k + 1) * C] = cT[k]
        bconst[:, CWT_O + k * C : CWT_O + (k + 1) * C] = cwT[k]

    fshared = np.zeros((1, FW), dtype=np.float32)
    present = np.zeros(C, dtype=bool)
    present[np.unique(t)] = True
    fshared[0, 0:C] = np.where(present, 0.0, PEN)

    xT = np.ascontiguousarray(x.T)                      # [D, B]

    in_maps = []
    for i in range(NCORES):
        rows = slice(i * ROWS, (i + 1) * ROWS)
        bcst = bconst.copy()
        # [m, p, k*128+a]: anchor-chunk-major packing of x.T
        xr = xT[:, rows].reshape(KD, 128, MCH, 128).transpose(2, 1, 0, 3)
        for m in range(MCH):
            bcst[:, XT_O + m * KD * 128 : XT_O + (m + 1) * KD * 128] = (
                xr[m].reshape(128, KD * 128)
            )
        ts = t[rows].astype(np.float32).reshape(MCH, 128)
        bcst[:, T_O : T_O + MCH] = ts.T
        in_maps.append({
            "bin": bcst.astype(ml_dtypes.bfloat16),
            "fin": fshared,
        })
    return in_maps


def kernel(inputs, centers, centers_weights, targets, epoch_number=None,
           **_ignored):
    nc = _get_nc()
    in_maps = make_in_maps(inputs, centers, centers_weights, targets)
    res = run_bass_kernel_spmd(nc, in_maps, core_ids=list(range(NCORES)))
    total = sum(float(r["out"][0, 0]) for r in res.results)
    return np.float32(total / B)



# revision 7
# speedup vs baseline: 1.1546x; 1.1546x over previous
"""Trainium2 Bass kernel for the AMTCL loss (nn_AMTCL_66520453480770).

Math: the reference's [B,B] pairwise-distance mining collapses to the [B,C]
matrix dc2[i,c] = sum_d w2[c,d]*(centers[c,d]-inputs[i,d])**2 because
dist[i,j] depends on j only through c = targets[j]:
    ap2[i] = dc2[i, t_i]
    an2[i] = min_{c present, c != t_i} dc2[i,c]
    cc2[i] = cdmin2[t_i],  cdmin2[c] = max(min_{j != c} cd2[c,j], 0)
    loss_i = sqrt(ap2) + relu(cc - an)
           = sqrt(ap2) + sqrt(cc2) - sqrt(min(an2, cc2))   (sqrt monotone)

Device work per core (512 anchors): one GEMM chain into PSUM
    dc2 = xsq @ w2.T + x @ m2.T + arow (rank-1) + PEN_OH * onehot
where the one-hot penalty is injected by a matmul with the TRANSPOSED
one-hot (lhsT=ohT chunk, rhs=PEN_OH*I), so per-chunk mining is just two
DVE reduces straight out of PSUM:
    an2 = min_c(psum),  ap2 = max_c(psum) - PEN_OH
(PEN_OH=2^22 keeps f32 ulp at 0.5 -> ap2 exact to ~1e-4; the -PEN_OH rides
the final Sqrt activation's bias). Absent classes carry +PEN_ABS=2^20 baked
into arow (> any dc2, < PEN_OH so the max still finds the self column).
cd2 [C,C] runs on the same tables + PEN_OH*I via an eyepen matmul; cc2 is
gathered per anchor by 4 tiny matmuls (lhsT=ohT, rhs=cdmin2 as bf16 column).

Host prep is O(C*D) / index-only: w2=2**cw, m2=-2*w2*c (bf16 tables),
a-row, one-hot transposed masks, x transpose/cast. Host also does the final
unshard: sum the [128,12] per-core outputs (cols 0:8 positive sqrt terms,
8:12 negative) and divide by B.

Schedule: DMA descriptor generation is split across the two HWDGE queues
(sync: tables+mask+arow, scalar: x halves) so descgen runs in parallel;
the scalar engine needs only the sqrt activation table (square/relu/sqrt
share one set), loaded once at T0 via a dummy sqrt; wide warmup matmuls
keep the PE p-state ramping until real data lands.
"""

import ml_dtypes
import numpy as np

import concourse.bass as bass
import concourse.bacc as bacc
import concourse.mybir as mybir
import concourse.tile as tile
from concourse.bass_utils import run_bass_kernel_spmd

B, C, D = 4096, 100, 384
NCORES = 8
ROWS = B // NCORES          # 512 anchor rows per core
MCH = ROWS // 128           # 4 partition chunks of anchor rows
KD = D // 128               # 3 partition chunks of the feature dim
PEN_OH = float(2 ** 22)     # one-hot / diagonal penalty (rides sqrt bias)
PEN_ABS = float(2 ** 20)    # absent-class penalty (baked into arow)
F32 = mybir.dt.float32
BF16 = mybir.dt.bfloat16
AF = mybir.ActivationFunctionType
ALU = mybir.AluOpType

NWARM = 5                   # [128,512] warmup matmuls before data lands

# ctab column layout (bf16): w2T | m2T | cT, each KD chunks of C cols
W2_O, M2_O, CT_O = 0, KD * C, 2 * KD * C
CTW = 3 * KD * C            # 900


def build_nc() -> bass.Bass:
    nc = bacc.Bacc(
        "TRN2", target_bir_lowering=False, debug=False, num_devices=NCORES
    )

    ctab_d = nc.declare_dram_parameter("ctab", [128, CTW], BF16, isOutput=False)
    mask_d = nc.declare_dram_parameter("mask", [128, MCH * 128], BF16,
                                       isOutput=False)
    arow_d = nc.declare_dram_parameter("arow", [4, C], BF16, isOutput=False)
    x_d = nc.declare_dram_parameter("x", [128, MCH * D], BF16, isOutput=False)
    out_d = nc.declare_dram_parameter("out", [128, 12], F32, isOutput=True)

    with tile.TileContext(nc) as tc:
        with (
            tc.tile_pool(name="wts", bufs=1) as wp,
            tc.tile_pool(name="ps1", bufs=1, space="PSUM") as pp1,
            tc.tile_pool(name="ps2", bufs=1, space="PSUM") as pp2,
        ):
            XH = MCH * D // 2
            # ---- DMAs: descgen split across the two HWDGE queues ----
            ctab = wp.tile([128, CTW], BF16, tag="ctab")
            nc.sync.dma_start(ctab[:], ctab_d[:])
            arow = wp.tile([4, C], BF16, tag="arow")
            nc.sync.dma_start(arow[:], arow_d[:])
            ohT = wp.tile([128, MCH * 128], BF16, tag="ohT")
            nc.sync.dma_start(ohT[:], mask_d[:])
            xsb = wp.tile([128, MCH * D], BF16, tag="xsb")
            nc.scalar.dma_start(xsb[:, 0:XH], x_d[:, 0:XH])
            nc.scalar.dma_start(xsb[:, XH:], x_d[:, XH:])

            w2t = ctab[:, W2_O : W2_O + KD * C]
            m2t = ctab[:, M2_O : M2_O + KD * C]
            ctt = ctab[:, CT_O : CT_O + KD * C]

            # ---- gpsimd: constants (no input deps) ----
            warm_b = wp.tile([128, 512], BF16, tag="warm_b")
            nc.gpsimd.memset(warm_b[:], 1.0)
            ones4 = wp.tile([4, 128], BF16, tag="ones4")
            nc.gpsimd.memset(ones4[:], 1.0)
            dums = wp.tile([1, 1], F32, tag="dums")
            nc.gpsimd.memset(dums[:], 1.0)
            negpen = wp.tile([128, 1], F32, tag="negpen")
            nc.gpsimd.memset(negpen[:], -PEN_OH)
            penb = wp.tile([C, C], BF16, tag="penb")
            nc.gpsimd.memset(penb[:], PEN_OH)
            eye_b = wp.tile([C, C], BF16, tag="eye_b")
            nc.gpsimd.affine_select(
                eye_b[:], warm_b[0:C, 0:C], pattern=[[1, C]],
                compare_op=ALU.is_equal, fill=0.0, base=0,
                channel_multiplier=-1,
            )
            eyepen_b = wp.tile([C, C], BF16, tag="eyepen_b")
            nc.gpsimd.affine_select(
                eyepen_b[:], penb[:], pattern=[[1, C]],
                compare_op=ALU.is_equal, fill=0.0, base=0,
                channel_multiplier=-1,
            )

            # ---- scalar: sqrt-table preload (square/relu/sqrt one set) ----
            dumsq = wp.tile([1, 1], F32, tag="dumsq")
            nc.scalar.sqrt(dumsq[:], dums[:])

            # ---- PE: p-state warmup until real operands land ----
            warm_ps = pp1.tile([128, 512], F32, tag="warm")
            for i in range(NWARM):
                nc.tensor.matmul(
                    warm_ps[:], warm_b[:, 0:128], warm_b[:],
                    start=(i == 0), stop=(i == NWARM - 1),
                )

            # ---- DVE: center squares (cd2 quad term rhs) ----
            csqt = wp.tile([128, KD * C], BF16, tag="csqt")
            nc.vector.tensor_tensor(csqt[:], ctt, ctt, op=ALU.mult)

            # ---- PE: cd2 [C,C] (cross+quad+diag now; arow rank-1 later) ----
            psum_cd2 = pp1.tile([C, C], F32, tag="cd2")
            for k in range(KD):
                nc.tensor.matmul(
                    psum_cd2[:], m2t[:, k * C : (k + 1) * C],
                    ctt[:, k * C : (k + 1) * C],
                    start=(k == 0), stop=False,
                )
            for k in range(KD):
                nc.tensor.matmul(
                    psum_cd2[:], w2t[:, k * C : (k + 1) * C],
                    csqt[:, k * C : (k + 1) * C],
                    start=False, stop=False,
                )
            nc.tensor.matmul(
                psum_cd2[:], eyepen_b[:], eye_b[:], start=False, stop=False,
            )

            # ---- main GEMM chunks 0..1 + cd2 finish + gathers + 2..3 ----
            an2all = wp.tile([128, MCH], F32, tag="an2all")
            tail = wp.tile([128, 12], F32, tag="tail")
            xsq = wp.tile([128, MCH * D], BF16, tag="xsq")
            psum_dc2 = []
            for m in range(MCH):
                psum_dc2.append(
                    pp2.tile([128, C], F32, name=f"dc2_{m}", tag=f"dc2_{m}")
                )

            def chunk_mms(m):
                xm = xsb[:, m * D : (m + 1) * D]
                xqm = xsq[:, m * D : (m + 1) * D]
                pd = psum_dc2[m]
                for k in range(KD):
                    nc.tensor.matmul(
                        pd[:], xm[:, k * 128 : (k + 1) * 128],
                        m2t[:, k * C : (k + 1) * C],
                        start=(k == 0), stop=False,
                    )
                for k in range(KD):
                    nc.tensor.matmul(
                        pd[:], xqm[:, k * 128 : (k + 1) * 128],
                        w2t[:, k * C : (k + 1) * C],
                        start=False, stop=False,
                    )
                nc.tensor.matmul(
                    pd[:], ohT[0:C, m * 128 : (m + 1) * 128], eyepen_b[:],
                    start=False, stop=False,
                )
                nc.tensor.matmul(
                    pd[:], ones4[:, 0:128], arow[:],
                    start=False, stop=True,
                )

            def mine(m):
                nc.vector.tensor_reduce(
                    an2all[:, m : m + 1], psum_dc2[m][:],
                    axis=mybir.AxisListType.X, op=ALU.min,
                )
                nc.vector.tensor_reduce(
                    tail[:, m : m + 1], psum_dc2[m][:],
                    axis=mybir.AxisListType.X, op=ALU.max,
                )

            # squares: chunk 0/1 on DVE early, 2 on scalar, 3 on DVE late
            def sq_dve(m):
                nc.vector.tensor_tensor(
                    xsq[:, m * D : (m + 1) * D], xsb[:, m * D : (m + 1) * D],
                    xsb[:, m * D : (m + 1) * D], op=ALU.mult,
                )

            sq_dve(0)
            sq_dve(1)
            nc.scalar.square(xsq[:, 2 * D : 3 * D], xsb[:, 2 * D : 3 * D])

            chunk_mms(0)
            # cd2 arow rank-1 (arow DMA lands slightly later)
            nc.tensor.matmul(
                psum_cd2[:], arow[:], ones4[:, 0:C], start=False, stop=True,
            )
            chunk_mms(1)

            mine(0)
            # DVE: cd2 min chain
            cdmin2 = wp.tile([C, 1], F32, tag="cdmin2")
            nc.vector.tensor_reduce(
                cdmin2[:], psum_cd2[:], axis=mybir.AxisListType.X, op=ALU.min
            )
            cdminb = wp.tile([C, 1], BF16, tag="cdminb")
            nc.vector.tensor_scalar(cdminb[:], cdmin2[:], 0.0, None,
                                    op0=ALU.max)
            mine(1)

            # PE: cc2 gathers (lhsT=ohT chunk, rhs=cdmin2 bf16 column)
            psum_cc2 = pp1.tile([128, MCH], F32, tag="cc2")
            for m in range(MCH):
                nc.tensor.matmul(
                    psum_cc2[:, m : m + 1],
                    ohT[0:C, m * 128 : (m + 1) * 128], cdminb[:],
                    start=True, stop=True,
                )

            sq_dve(3)
            chunk_mms(2)
            chunk_mms(3)
            mine(2)
            mine(3)

            # ---- tail: loss_i = sqrt(ap2) + sqrt(cc2) - sqrt(min) ----
            nc.vector.tensor_copy(tail[:, 4:8], psum_cc2[:])
            nc.vector.tensor_tensor(
                tail[:, 8:12], an2all[:], psum_cc2[:], op=ALU.min
            )
            tailsq = wp.tile([128, 12], F32, tag="tailsq")
            nc.scalar.activation(tailsq[:, 0:4], tail[:, 0:4], AF.Sqrt,
                                 bias=negpen[:])
            nc.scalar.activation(tailsq[:, 4:12], tail[:, 4:12], AF.Sqrt)
            nc.scalar.dma_start(out_d[:], tailsq[:])

    nc.compile()
    return nc


_NC_CACHE: list = []


def _get_nc() -> bass.Bass:
    if not _NC_CACHE:
        _NC_CACHE.append(build_nc())
    return _NC_CACHE[0]


def make_in_maps(inputs, centers, centers_weights, targets):
    x = np.asarray(inputs, dtype=np.float32)
    c = np.asarray(centers, dtype=np.float32)
    cw = np.asarray(centers_weights, dtype=np.float32)
    t = np.asarray(targets).astype(np.int64)
    bf = ml_dtypes.bfloat16

    w2 = (2.0 ** cw).astype(np.float32)                 # [C, D]
    m2 = -2.0 * w2 * c                                  # [C, D]

    ctab = np.zeros((128, CTW), dtype=np.float32)
    for k in range(KD):
        sl = slice(k * 128, (k + 1) * 128)
        ctab[:, W2_O + k * C : W2_O + (k + 1) * C] = w2.T[sl]
        ctab[:, M2_O + k * C : M2_O + (k + 1) * C] = m2.T[sl]
        ctab[:, CT_O + k * C : CT_O + (k + 1) * C] = c.T[sl]
    ctab = ctab.astype(bf)

    present = np.zeros(C, dtype=bool)
    present[np.unique(t)] = True
    a = (w2 * c * c).sum(axis=1) + PEN_ABS * (~present)
    arow = np.zeros((4, C), dtype=np.float32)
    arow[0] = a
    arow = arow.astype(bf)

    xT = np.ascontiguousarray(x.T)                      # [D, B]

    in_maps = []
    for i in range(NCORES):
        rows = slice(i * ROWS, (i + 1) * ROWS)
        # [m, p, k*128+j]: anchor-chunk-major packing of x.T
        xr = xT[:, rows].reshape(KD, 128, MCH, 128).transpose(2, 1, 0, 3)
        xd = xr.reshape(MCH * 128, KD * 128).reshape(MCH, 128, KD * 128)
        xd = np.concatenate([xd[m] for m in range(MCH)], axis=1)
        ts = t[rows].reshape(MCH, 128)
        ohT = np.zeros((128, MCH * 128), dtype=np.float32)
        for m in range(MCH):
            ohT[:C, m * 128 : (m + 1) * 128] = (
                np.arange(C)[:, None] == ts[m][None, :]
            )
        in_maps.append({
            "ctab": ctab,
            "mask": ohT.astype(bf),
            "arow": arow,
            "x": xd.astype(bf),
        })
    return in_maps


def kernel(inputs, centers, centers_weights, targets, epoch_number=None,
           **_ignored):
    nc = _get_nc()
    in_maps = make_in_maps(inputs, centers, centers_weights, targets)
    res = run_bass_kernel_spmd(nc, in_maps, core_ids=list(range(NCORES)))
    total = 0.0
    for r in res.results:
        o = np.asarray(r["out"], dtype=np.float64)
        total += o[:, 0:8].sum() - o[:, 8:12].sum()
    return np.float32(total / B)


# revision 12
# speedup vs baseline: 1.2825x; 1.1108x over previous
"""Trainium2 Bass kernel for the AMTCL loss (nn_AMTCL_66520453480770).

Math: the reference's [B,B] pairwise-distance mining collapses to the [B,C]
matrix dc2[i,c] = sum_d w2[c,d]*(centers[c,d]-inputs[i,d])**2 because
dist[i,j] depends on j only through c = targets[j]:
    ap2[i] = dc2[i, t_i]
    an2[i] = min_{c present, c != t_i} dc2[i,c]
    cc2[i] = cdmin2[t_i],  cdmin2[c] = max(min_{j != c} cd2[c,j], 0)
    loss_i = sqrt(ap2) + relu(cc - an)
           = sqrt(ap2) + sqrt(cc2) - sqrt(min(an2, cc2))   (sqrt monotone)

Device GEMM chain per 128-anchor chunk (PSUM f32):
    dc2 = xsq @ w2.T + x @ m2.T + [PEN_OH*onehot + arow]
where the bracket is ONE matmul: lhsT = [ohT; ones-row] (fp8, 101 x 128),
rhs = epa = [PEN_OH*I; arow] (bf16, 101 x 100).  Mining is then just two DVE
reduces straight out of PSUM: an2 = min_c, ap2 = max_c - PEN_OH (PEN_OH=2^22
keeps f32 ulp at 0.5; the -PEN_OH rides the final Sqrt activation's bias).
Absent classes carry +PEN_ABS=2^20 inside arow (> any dc2, < PEN_OH so the
max still finds the self column). cd2 [C,C] reuses the tables plus one
[eyepen;arow] x [eye;ones] matmul; cc2 is gathered per anchor chunk by a
tiny matmul (lhsT=ohT fp8, rhs=cdmin2 as bf16 column).

DMA layout is descriptor-economical (the HW DGE costs ~70ns per partition
row regardless of size): 3 input DMAs total — tabs bf16 [128,900]
(w2T|m2T|cT), xoh fp8 [128,2048] (x chunks | ohT+ones-row) split in two
starts for pipelining, and a 1-descriptor arow row landing directly in
partition 100 of the epa tile. Descriptor generation runs in parallel on
the sync and scalar HWDGE queues. x in fp8 (e4m3) halves bytes; squares of
fp8 are exact in bf16; mixed fp8-lhsT x bf16-rhs matmuls are exact.

The scalar engine needs only the sqrt activation-table set (square, relu,
sqrt share one), loaded once at body start via a dummy sqrt. Wide warmup
matmuls keep the PE busy until data lands (the PE clocks up from 1.2 to
2.4 GHz only after ~3.5us of uninterrupted work).

Host work is O(C*D) table prep / index packing plus the final unshard:
sum the [128,12] per-core outputs (cols 0:8 positive, 8:12 negative), /B.
"""

import ml_dtypes
import numpy as np

import concourse.bass as bass
import concourse.bacc as bacc
import concourse.mybir as mybir
import concourse.tile as tile
from concourse.bass_utils import run_bass_kernel_spmd

B, C, D = 4096, 100, 384
NCORES = 8
ROWS = B // NCORES          # 512 anchor rows per core
MCH = ROWS // 128           # 4 partition chunks of anchor rows
KD = D // 128               # 3 partition chunks of the feature dim
PEN_OH = float(2 ** 22)     # one-hot / diagonal penalty (rides sqrt bias)
PEN_ABS = float(2 ** 20)    # absent-class penalty (baked into arow)
F32 = mybir.dt.float32
BF16 = mybir.dt.bfloat16
FP8 = mybir.dt.float8e4
AF = mybir.ActivationFunctionType
ALU = mybir.AluOpType

NWARM = 4                   # [128,512] warmup matmuls before data lands

# tabs column layout (bf16): w2T | m2T | cT, each KD chunks of C cols
W2_O, M2_O, CT_O = 0, KD * C, 2 * KD * C
TABW = 3 * KD * C           # 900
# xoh column layout (fp8): x chunks (m-major) | ohT (+ones row 100)
XW = MCH * D                # 1536
OH_O = XW
XOHW = XW + MCH * 128       # 2048
XSPLIT = 2 * D              # first xoh DMA covers chunks 0..1


def build_nc() -> bass.Bass:
    nc = bacc.Bacc(
        "TRN2", target_bir_lowering=False, debug=False, num_devices=NCORES
    )

    tabs_d = nc.declare_dram_parameter("tabs", [128, TABW], BF16,
                                       isOutput=False)
    xoh_d = nc.declare_dram_parameter("xoh", [128, XOHW], FP8, isOutput=False)
    arow_d = nc.declare_dram_parameter("arow", [2, C], BF16, isOutput=False)
    out_d = nc.declare_dram_parameter("out", [128, 12], F32, isOutput=True)

    with tile.TileContext(nc) as tc:
        with (
            tc.tile_pool(name="wts", bufs=1) as wp,
            tc.tile_pool(name="ps1", bufs=1, space="PSUM") as pp1,
            tc.tile_pool(name="ps2", bufs=1, space="PSUM") as pp2,
        ):
            # ---- DMAs: descgen split across the two HWDGE queues ----
            tabs = wp.tile([128, TABW], BF16, tag="tabs")
            nc.sync.dma_start(tabs[:], tabs_d[:])
            epa = wp.tile([101, C], BF16, tag="epa")
            nc.sync.dma_start(epa[100:101, :], arow_d[0:1, :])
            xoh = wp.tile([128, XOHW], FP8, tag="xoh")
            nc.scalar.dma_start(xoh[:, 0:XSPLIT], xoh_d[:, 0:XSPLIT])
            nc.scalar.dma_start(xoh[:, XSPLIT:], xoh_d[:, XSPLIT:])

            w2t = tabs[:, W2_O : W2_O + KD * C]
            m2t = tabs[:, M2_O : M2_O + KD * C]
            ctt = tabs[:, CT_O : CT_O + KD * C]

            # ---- gpsimd: constants (no input deps) ----
            warm_b = wp.tile([128, 512], BF16, tag="warm_b")
            nc.gpsimd.memset(warm_b[:], 1.0)
            dums = wp.tile([1, 1], F32, tag="dums")
            nc.gpsimd.memset(dums[:], 1.0)
            negpen = wp.tile([128, 1], F32, tag="negpen")
            nc.gpsimd.memset(negpen[:], -PEN_OH)
            penb = wp.tile([C, C], BF16, tag="penb")
            nc.gpsimd.memset(penb[:], PEN_OH)
            eyeone = wp.tile([101, C], BF16, tag="eyeone")
            nc.sync.dma_start(eyeone[100:101, :], arow_d[1:2, :])
            nc.gpsimd.affine_select(
                eyeone[0:C, :], warm_b[0:C, 0:C], pattern=[[1, C]],
                compare_op=ALU.is_equal, fill=0.0, base=0,
                channel_multiplier=-1,
            )
            nc.gpsimd.affine_select(
                epa[0:C, :], penb[:], pattern=[[1, C]],
                compare_op=ALU.is_equal, fill=0.0, base=0,
                channel_multiplier=-1,
            )
            # center squares for the cd2 quad term (gpsimd: DVE is busy)
            csqt = wp.tile([128, KD * C], BF16, tag="csqt")
            nc.gpsimd.tensor_tensor(csqt[:], ctt, ctt, op=ALU.mult)

            # ---- scalar: sqrt-table preload (square/relu/sqrt one set) ----
            dumsq = wp.tile([1, 1], F32, tag="dumsq")
            nc.scalar.sqrt(dumsq[:], dums[:])

            # ---- PE: p-state warmup until real operands land ----
            warm_ps = pp1.tile([128, 512], F32, tag="warm")
            for i in range(NWARM):
                nc.tensor.matmul(
                    warm_ps[:], warm_b[:, 0:128], warm_b[:],
                    start=(i == 0), stop=(i == NWARM - 1),
                )

            xsq = wp.tile([128, XW], BF16, tag="xsq")

            def sq(eng, m):
                sl = slice(m * D, (m + 1) * D)
                if eng is nc.vector:
                    eng.tensor_tensor(xsq[:, sl], xoh[:, sl], xoh[:, sl],
                                      op=ALU.mult)
                else:
                    eng.square(xsq[:, sl], xoh[:, sl])

            psum_dc2 = []
            for m in range(MCH):
                psum_dc2.append(
                    pp2.tile([128, C], F32, name=f"dc2_{m}", tag=f"dc2_{m}")
                )

            def chunk_x_mms(m):
                for k in range(KD):
                    nc.tensor.matmul(
                        psum_dc2[m][:],
                        xoh[:, m * D + k * 128 : m * D + (k + 1) * 128],
                        m2t[:, k * C : (k + 1) * C],
                        start=(k == 0), stop=False,
                    )

            def chunk_xsq_mms(m):
                for k in range(KD):
                    nc.tensor.matmul(
                        psum_dc2[m][:],
                        xsq[:, m * D + k * 128 : m * D + (k + 1) * 128],
                        w2t[:, k * C : (k + 1) * C],
                        start=False, stop=False,
                    )

            def chunk_pen_mm(m):
                nc.tensor.matmul(
                    psum_dc2[m][:],
                    xoh[0:101, OH_O + m * 128 : OH_O + (m + 1) * 128],
                    epa[:], start=False, stop=True,
                )

            an2all = wp.tile([128, MCH], F32, tag="an2all")
            tail = wp.tile([128, 12], F32, tag="tail")

            def mine(m):
                nc.vector.tensor_reduce(
                    an2all[:, m : m + 1], psum_dc2[m][:],
                    axis=mybir.AxisListType.X, op=ALU.min,
                )
                nc.vector.tensor_reduce(
                    tail[:, m : m + 1], psum_dc2[m][:],
                    axis=mybir.AxisListType.X, op=ALU.max,
                )

            # ---- squares: chunks 0/1 on DVE, 2/3 on scalar ----
            sq(nc.vector, 0)
            sq(nc.vector, 1)
            sq(nc.scalar, 2)
            sq(nc.scalar, 3)

            # ---- PE stream (ready-order to avoid FIFO stalls) ----
            psum_cd2 = pp1.tile([C, C], F32, tag="cd2")
            chunk_x_mms(0)
            for k in range(KD):          # cd2 cross terms (tabs only)
                nc.tensor.matmul(
                    psum_cd2[:], m2t[:, k * C : (k + 1) * C],
                    ctt[:, k * C : (k + 1) * C],
                    start=(k == 0), stop=False,
                )
            chunk_xsq_mms(0)
            chunk_pen_mm(0)
            chunk_x_mms(1)
            chunk_xsq_mms(1)
            chunk_pen_mm(1)
            for k in range(KD):          # cd2 quad terms (need csqt)
                nc.tensor.matmul(
                    psum_cd2[:], w2t[:, k * C : (k + 1) * C],
                    csqt[:, k * C : (k + 1) * C],
                    start=False, stop=False,
                )
            nc.tensor.matmul(            # cd2 diag penalty + arow rank-1
                psum_cd2[:], epa[:], eyeone[:], start=False, stop=True,
            )
            chunk_x_mms(2)
            chunk_xsq_mms(2)
            chunk_pen_mm(2)
            chunk_x_mms(3)
            chunk_xsq_mms(3)
            chunk_pen_mm(3)

            # ---- DVE: mining + cd2 min chain ----
            mine(0)
            mine(1)
            cdmin2 = wp.tile([C, 1], F32, tag="cdmin2")
            nc.vector.tensor_reduce(
                cdmin2[:], psum_cd2[:], axis=mybir.AxisListType.X, op=ALU.min
            )
            cdminb = wp.tile([C, 1], BF16, tag="cdminb")
            nc.vector.tensor_scalar(cdminb[:], cdmin2[:], 0.0, None,
                                    op0=ALU.max)
            mine(2)
            mine(3)

            # ---- PE: cc2 gathers (lhsT=ohT fp8, rhs=cdmin2 bf16 col) ----
            psum_cc2 = pp1.tile([128, MCH], F32, tag="cc2")
            for m in range(MCH):
                nc.tensor.matmul(
                    psum_cc2[:, m : m + 1],
                    xoh[0:C, OH_O + m * 128 : OH_O + (m + 1) * 128],
                    cdminb[:], start=True, stop=True,
                )

            # ---- tail: loss_i = sqrt(ap2) + sqrt(cc2) - sqrt(min) ----
            nc.vector.tensor_copy(tail[:, 4:8], psum_cc2[:])
            nc.vector.tensor_tensor(
                tail[:, 8:12], an2all[:], psum_cc2[:], op=ALU.min
            )
            tailsq = wp.tile([128, 12], F32, tag="tailsq")
            nc.scalar.activation(tailsq[:, 0:4], tail[:, 0:4], AF.Sqrt,
                                 bias=negpen[:])
            nc.scalar.activation(tailsq[:, 4:12], tail[:, 4:12], AF.Sqrt)
            nc.scalar.dma_start(out_d[:], tailsq[:])

    nc.compile()
    return nc


_NC_CACHE: list = []


def _get_nc() -> bass.Bass:
    if not _NC_CACHE:
        _NC_CACHE.append(build_nc())
    return _NC_CACHE[0]


def make_in_maps(inputs, centers, centers_weights, targets):
    x = np.asarray(inputs, dtype=np.float32)
    c = np.asarray(centers, dtype=np.float32)
    cw = np.asarray(centers_weights, dtype=np.float32)
    t = np.asarray(targets).astype(np.int64)
    bf = ml_dtypes.bfloat16
    f8 = ml_dtypes.float8_e4m3

    w2 = (2.0 ** cw).astype(np.float32)                 # [C, D]
    m2 = -2.0 * w2 * c                                  # [C, D]

    tabs = np.zeros((128, TABW), dtype=np.float32)
    for k in range(KD):
        sl = slice(k * 128, (k + 1) * 128)
        tabs[:, W2_O + k * C : W2_O + (k + 1) * C] = w2.T[sl]
        tabs[:, M2_O + k * C : M2_O + (k + 1) * C] = m2.T[sl]
        tabs[:, CT_O + k * C : CT_O + (k + 1) * C] = c.T[sl]
    tabs = tabs.astype(bf)

    present = np.zeros(C, dtype=bool)
    present[np.unique(t)] = True
    arow = np.ones((2, C), dtype=np.float32)            # row 1: ones row
    arow[0] = (w2 * c * c).sum(axis=1) + PEN_ABS * (~present)
    arow = arow.astype(bf)

    xT = np.ascontiguousarray(x.T)                      # [D, B]

    in_maps = []
    for i in range(NCORES):
        rows = slice(i * ROWS, (i + 1) * ROWS)
        xoh = np.zeros((128, XOHW), dtype=np.float32)
        # [m, p, k*128+j]: anchor-chunk-major packing of x.T
        xr = xT[:, rows].reshape(KD, 128, MCH, 128).transpose(2, 1, 0, 3)
        xoh[:, 0:XW] = xr.reshape(MCH, 128, KD * 128).transpose(
            1, 0, 2).reshape(128, XW)
        ts = t[rows].reshape(MCH, 128)
        for m in range(MCH):
            xoh[:C, OH_O + m * 128 : OH_O + (m + 1) * 128] = (
                np.arange(C)[:, None] == ts[m][None, :]
            )
        xoh[C:101, OH_O:] = 0.0
        xoh[100, OH_O:] = 1.0                           # arow ones row
        in_maps.append({
            "tabs": tabs,
            "xoh": xoh.astype(f8),
            "arow": arow,
        })
    return in_maps


def kernel(inputs, centers, centers_weights, targets, epoch_number=None,
           **_ignored):
    nc = _get_nc()
    in_maps = make_in_maps(inputs, centers, centers_weights, targets)
    res = run_bass_kernel_spmd(nc, in_maps, core_ids=list(range(NCORES)))
    total = 0.0
    for r in res.results:
        o = np.asarray(r["out"], dtype=np.float64)
        total += o[:, 0:8].sum() - o[:, 8:12].sum()
    return np.float32(total / B)


# revision 17
# speedup vs baseline: 1.3009x; 1.0144x over previous
"""Trainium2 Bass kernel for the AMTCL loss (nn_AMTCL_66520453480770).

Math: the reference's [B,B] pairwise-distance mining collapses to the [B,C]
matrix dc2[i,c] = sum_d w2[c,d]*(centers[c,d]-inputs[i,d])**2 because
dist[i,j] depends on j only through c = targets[j]:
    ap2[i] = dc2[i, t_i]
    an2[i] = min_{c present, c != t_i} dc2[i,c]
    cc2[i] = cdmin2[t_i],  cdmin2[c] = max(min_{j != c} cd2[c,j], 0)
    loss_i = sqrt(ap2) + relu(cc - an)
           = sqrt(ap2) + sqrt(cc2) - sqrt(min(an2, cc2))   (sqrt monotone)

Device GEMM chain per 128-anchor chunk (PSUM f32):
    dc2 = xsq @ w2.T + x @ m2.T + [PEN_OH*onehot + arow]
where the bracket is ONE matmul: lhsT = [ohT; ones-row] (fp8, 101 x 128),
rhs = epa = [PEN_OH*I; arow] (bf16, 101 x 100).  Mining is then just two DVE
reduces straight out of PSUM: an2 = min_c, ap2 = max_c - PEN_OH (PEN_OH=2^22
keeps f32 ulp at 0.5; the -PEN_OH rides the final Sqrt activation's bias).
Absent classes carry +PEN_ABS=2^20 inside arow (> any dc2, < PEN_OH so the
max still finds the self column). cd2 [C,C] reuses the tables plus one
[eyepen;arow] x [eye;ones] matmul; cc2 is gathered per anchor chunk by a
tiny matmul (lhsT=ohT fp8, rhs=cdmin2 as bf16 column).

DMA layout is descriptor-economical (the HW DGE costs ~70ns per partition
row regardless of size): 3 input DMAs total — tabs bf16 [128,900]
(w2T|m2T|cT), xoh fp8 [128,2048] (x chunks | ohT+ones-row) split in two
starts for pipelining, and a 1-descriptor arow row landing directly in
partition 100 of the epa tile. Descriptor generation runs in parallel on
the sync and scalar HWDGE queues. x in fp8 (e4m3) halves bytes; squares of
fp8 are exact in bf16; mixed fp8-lhsT x bf16-rhs matmuls are exact.

The scalar engine needs only the sqrt activation-table set (square, relu,
sqrt share one), loaded once at body start via a dummy sqrt. Wide warmup
matmuls keep the PE busy until data lands (the PE clocks up from 1.2 to
2.4 GHz only after ~3.5us of uninterrupted work).

Host work is O(C*D) table prep / index packing plus the final unshard:
sum the [128,12] per-core outputs (cols 0:8 positive, 8:12 negative), /B.
"""

import ml_dtypes
import numpy as np

import concourse.bass as bass
import concourse.bacc as bacc
import concourse.mybir as mybir
import concourse.tile as tile
from concourse.bass_utils import run_bass_kernel_spmd

B, C, D = 4096, 100, 384
NCORES = 8
ROWS = B // NCORES          # 512 anchor rows per core
MCH = ROWS // 128           # 4 partition chunks of anchor rows
KD = D // 128               # 3 partition chunks of the feature dim
PEN_OH = float(2 ** 22)     # one-hot / diagonal penalty (rides sqrt bias)
PEN_ABS = float(2 ** 20)    # absent-class penalty (baked into arow)
F32 = mybir.dt.float32
BF16 = mybir.dt.bfloat16
FP8 = mybir.dt.float8e4
AF = mybir.ActivationFunctionType
ALU = mybir.AluOpType

NWARM = 6                   # [128,512] warmup matmuls before data lands

# tabs column layout (bf16): w2T | m2T | cT, each KD chunks of C cols
W2_O, M2_O, CT_O = 0, KD * C, 2 * KD * C
TABW = 3 * KD * C           # 900
# xoh column layout (fp8): ohT (+ones row 100) | x chunks (m-major)
OH_O = 0
X_O = MCH * 128             # 512
XOHW = X_O + MCH * D        # 2048
XSPLIT = X_O + 2 * D        # first xoh DMA covers ohT + chunks 0..1


def build_nc() -> bass.Bass:
    nc = bacc.Bacc(
        "TRN2", target_bir_lowering=False, debug=False, num_devices=NCORES
    )

    tabs_d = nc.declare_dram_parameter("tabs", [128, TABW], BF16,
                                       isOutput=False)
    xoh_d = nc.declare_dram_parameter("xoh", [128, XOHW], FP8, isOutput=False)
    arow_d = nc.declare_dram_parameter("arow", [2, C], BF16, isOutput=False)
    out_d = nc.declare_dram_parameter("out", [128, 12], F32, isOutput=True)

    with tile.TileContext(nc) as tc:
        with (
            tc.tile_pool(name="wts", bufs=1) as wp,
            tc.tile_pool(name="ps1", bufs=1, space="PSUM") as pp1,
            tc.tile_pool(name="ps2", bufs=1, space="PSUM") as pp2,
        ):
            # ---- DMAs: descgen split across the two HWDGE queues ----
            tabs = wp.tile([128, TABW], BF16, tag="tabs")
            nc.sync.dma_start(tabs[:], tabs_d[:])
            epa = wp.tile([101, C], BF16, tag="epa")
            nc.sync.dma_start(epa[100:101, :], arow_d[0:1, :])
            xoh = wp.tile([128, XOHW], FP8, tag="xoh")
            nc.scalar.dma_start(xoh[:, 0:XSPLIT], xoh_d[:, 0:XSPLIT])
            nc.scalar.dma_start(xoh[:, XSPLIT:], xoh_d[:, XSPLIT:])

            w2t = tabs[:, W2_O : W2_O + KD * C]
            m2t = tabs[:, M2_O : M2_O + KD * C]
            ctt = tabs[:, CT_O : CT_O + KD * C]

            # ---- gpsimd: constants (no input deps) ----
            warm_b = wp.tile([128, 512], BF16, tag="warm_b")
            nc.gpsimd.memset(warm_b[:], 1.0)
            dums = wp.tile([1, 1], F32, tag="dums")
            nc.gpsimd.memset(dums[:], 1.0)
            negpen = wp.tile([128, 1], F32, tag="negpen")
            nc.gpsimd.memset(negpen[:], -PEN_OH)
            penb = wp.tile([C, C], BF16, tag="penb")
            nc.gpsimd.memset(penb[:], PEN_OH)
            eyeone = wp.tile([101, C], BF16, tag="eyeone")
            nc.sync.dma_start(eyeone[100:101, :], arow_d[1:2, :])
            nc.gpsimd.affine_select(
                eyeone[0:C, :], warm_b[0:C, 0:C], pattern=[[1, C]],
                compare_op=ALU.is_equal, fill=0.0, base=0,
                channel_multiplier=-1,
            )
            nc.gpsimd.affine_select(
                epa[0:C, :], penb[:], pattern=[[1, C]],
                compare_op=ALU.is_equal, fill=0.0, base=0,
                channel_multiplier=-1,
            )
            # center squares for the cd2 quad term (gpsimd: DVE is busy)
            csqt = wp.tile([128, KD * C], BF16, tag="csqt")
            nc.gpsimd.tensor_tensor(csqt[:], ctt, ctt, op=ALU.mult)

            # ---- scalar: sqrt-table preload (square/relu/sqrt one set) ----
            dumsq = wp.tile([1, 1], F32, tag="dumsq")
            nc.scalar.sqrt(dumsq[:], dums[:])

            # ---- PE: p-state warmup until real operands land ----
            warm_ps = pp1.tile([128, 512], F32, tag="warm")
            for i in range(NWARM):
                nc.tensor.matmul(
                    warm_ps[:], warm_b[:, 0:128], warm_b[:],
                    start=(i == 0), stop=(i == NWARM - 1),
                )

            xsq = wp.tile([128, MCH * D], BF16, tag="xsq")

            def sq(eng, m):
                xl = slice(X_O + m * D, X_O + (m + 1) * D)
                sl = slice(m * D, (m + 1) * D)
                if eng is nc.vector:
                    eng.tensor_tensor(xsq[:, sl], xoh[:, xl], xoh[:, xl],
                                      op=ALU.mult)
                else:
                    eng.square(xsq[:, sl], xoh[:, xl])

            psum_dc2 = []
            for m in range(MCH):
                psum_dc2.append(
                    pp2.tile([128, C], F32, name=f"dc2_{m}", tag=f"dc2_{m}")
                )

            def chunk_x_mms(m):
                for k in range(KD):
                    nc.tensor.matmul(
                        psum_dc2[m][:],
                        xoh[:, X_O + m * D + k * 128 :
                               X_O + m * D + (k + 1) * 128],
                        m2t[:, k * C : (k + 1) * C],
                        start=(k == 0), stop=False,
                    )

            def chunk_xsq_mms(m):
                for k in range(KD):
                    nc.tensor.matmul(
                        psum_dc2[m][:],
                        xsq[:, m * D + k * 128 : m * D + (k + 1) * 128],
                        w2t[:, k * C : (k + 1) * C],
                        start=False, stop=False,
                    )

            def chunk_pen_mm(m):
                nc.tensor.matmul(
                    psum_dc2[m][:],
                    xoh[0:101, OH_O + m * 128 : OH_O + (m + 1) * 128],
                    epa[:], start=False, stop=True,
                )

            an2all = wp.tile([128, MCH], F32, tag="an2all")
            tail = wp.tile([128, 12], F32, tag="tail")

            def mine(m):
                nc.vector.tensor_reduce(
                    an2all[:, m : m + 1], psum_dc2[m][:],
                    axis=mybir.AxisListType.X, op=ALU.min,
                )
                nc.vector.tensor_reduce(
                    tail[:, m : m + 1], psum_dc2[m][:],
                    axis=mybir.AxisListType.X, op=ALU.max,
                )

            # ---- squares: chunks 0/1 on DVE, 2/3 on scalar ----
            sq(nc.vector, 0)
            sq(nc.vector, 1)
            sq(nc.scalar, 2)
            sq(nc.scalar, 3)

            # ---- PE stream (ready-order to avoid FIFO stalls) ----
            psum_cd2 = pp1.tile([C, C], F32, tag="cd2")
            chunk_x_mms(0)
            for k in range(KD):          # cd2 cross terms (tabs only)
                nc.tensor.matmul(
                    psum_cd2[:], m2t[:, k * C : (k + 1) * C],
                    ctt[:, k * C : (k + 1) * C],
                    start=(k == 0), stop=False,
                )
            chunk_xsq_mms(0)
            chunk_pen_mm(0)
            chunk_x_mms(1)
            chunk_xsq_mms(1)
            chunk_pen_mm(1)
            for k in range(KD):          # cd2 quad terms (need csqt)
                nc.tensor.matmul(
                    psum_cd2[:], w2t[:, k * C : (k + 1) * C],
                    csqt[:, k * C : (k + 1) * C],
                    start=False, stop=False,
                )
            nc.tensor.matmul(            # cd2 diag penalty + arow rank-1
                psum_cd2[:], epa[:], eyeone[:], start=False, stop=True,
            )
            chunk_x_mms(2)
            chunk_xsq_mms(2)
            chunk_pen_mm(2)
            chunk_x_mms(3)
            chunk_xsq_mms(3)
            chunk_pen_mm(3)

            # ---- DVE: mining + cd2 min chain ----
            mine(0)
            mine(1)
            cdmin2 = wp.tile([C, 1], F32, tag="cdmin2")
            nc.vector.tensor_reduce(
                cdmin2[:], psum_cd2[:], axis=mybir.AxisListType.X, op=ALU.min
            )
            cdminb = wp.tile([C, 1], BF16, tag="cdminb")
            nc.vector.tensor_scalar(cdminb[:], cdmin2[:], 0.0, None,
                                    op0=ALU.max)
            mine(2)
            mine(3)

            # ---- PE: cc2 gathers (lhsT=ohT fp8, rhs=cdmin2 bf16 col) ----
            psum_cc2 = pp1.tile([128, MCH], F32, tag="cc2")
            for m in range(MCH):
                nc.tensor.matmul(
                    psum_cc2[:, m : m + 1],
                    xoh[0:C, OH_O + m * 128 : OH_O + (m + 1) * 128],
                    cdminb[:], start=True, stop=True,
                )

            # ---- tail: loss_i = sqrt(ap2) + sqrt(cc2) - sqrt(min) ----
            nc.vector.tensor_copy(tail[:, 4:8], psum_cc2[:])
            nc.vector.tensor_tensor(
                tail[:, 8:12], an2all[:], psum_cc2[:], op=ALU.min
            )
            tailsq = wp.tile([128, 12], F32, tag="tailsq")
            nc.scalar.activation(tailsq[:, 0:4], tail[:, 0:4], AF.Sqrt,
                                 bias=negpen[:])
            nc.scalar.activation(tailsq[:, 4:12], tail[:, 4:12], AF.Sqrt)
            nc.scalar.dma_start(out_d[:], tailsq[:])

    nc.compile()
    return nc


_NC_CACHE: list = []


def _get_nc() -> bass.Bass:
    if not _NC_CACHE:
        _NC_CACHE.append(build_nc())
    return _NC_CACHE[0]


def make_in_maps(inputs, centers, centers_weights, targets):
    x = np.asarray(inputs, dtype=np.float32)
    c = np.asarray(centers, dtype=np.float32)
    cw = np.asarray(centers_weights, dtype=np.float32)
    t = np.asarray(targets).astype(np.int64)
    bf = ml_dtypes.bfloat16
    f8 = ml_dtypes.float8_e4m3

    w2 = (2.0 ** cw).astype(np.float32)                 # [C, D]
    m2 = -2.0 * w2 * c                                  # [C, D]

    tabs = np.zeros((128, TABW), dtype=np.float32)
    for k in range(KD):
        sl = slice(k * 128, (k + 1) * 128)
        tabs[:, W2_O + k * C : W2_O + (k + 1) * C] = w2.T[sl]
        tabs[:, M2_O + k * C : M2_O + (k + 1) * C] = m2.T[sl]
        tabs[:, CT_O + k * C : CT_O + (k + 1) * C] = c.T[sl]
    tabs = tabs.astype(bf)

    present = np.zeros(C, dtype=bool)
    present[np.unique(t)] = True
    arow = np.ones((2, C), dtype=np.float32)            # row 1: ones row
    arow[0] = (w2 * c * c).sum(axis=1) + PEN_ABS * (~present)
    arow = arow.astype(bf)

    xT = np.ascontiguousarray(x.T)                      # [D, B]

    in_maps = []
    for i in range(NCORES):
        rows = slice(i * ROWS, (i + 1) * ROWS)
        xoh = np.zeros((128, XOHW), dtype=np.float32)
        # [m, p, k*128+j]: anchor-chunk-major packing of x.T
        xr = xT[:, rows].reshape(KD, 128, MCH, 128).transpose(2, 1, 0, 3)
        xoh[:, X_O:] = xr.reshape(MCH, 128, KD * 128).transpose(
            1, 0, 2).reshape(128, MCH * KD * 128)
        ts = t[rows].reshape(MCH, 128)
        for m in range(MCH):
            xoh[:C, OH_O + m * 128 : OH_O + (m + 1) * 128] = (
                np.arange(C)[:, None] == ts[m][None, :]
            )
        xoh[C:, OH_O : OH_O + MCH * 128] = 0.0
        xoh[100, OH_O : OH_O + MCH * 128] = 1.0         # arow ones row
        in_maps.append({
            "tabs": tabs,
            "xoh": xoh.astype(f8),
            "arow": arow,
        })
    return in_maps


def kernel(inputs, centers, centers_weights, targets, epoch_number=None,
           **_ignored):
    nc = _get_nc()
    in_maps = make_in_maps(inputs, centers, centers_weights, targets)
    res = run_bass_kernel_spmd(nc, in_maps, core_ids=list(range(NCORES)))
    total = 0.0
    for r in res.results:
        o = np.asarray(r["out"], dtype=np.float64)
        total += o[:, 0:8].sum() - o[:, 8:12].sum()
    return np.float32(total / B)
